# revision 6
# baseline (speedup 1.0000x reference)
"""Trainium2 Bass kernel for windowed 3D attention (nn_Attention_12927851561046).

512 windows of 343-token, 4-head, 32-dim-per-head attention over d=128.
Pure data parallel: 64 windows per core across 8 NeuronCores.

v2 layout strategy (per window):
  xt (d=128 partitions, 384 tokens padded) bf16
  q/k/v projections -> psum scratch (2 rotating banks) -> bf16 sbuf casts (DVE)
  v cast lands in vext const tiles laid out [v_h0|1|1|v_h1] per (c, head-pair)
  sim^T per (c, head-pair): 2 matmuls into one 2-bank psum tile (j part., i free)
  exp on ACT: one instr per pair, strided 2-bank read -> packed bf16 sbuf
  attn = expsim * exp(bias) on Pool (gpsimd, sbuf-only)
  AV fused with rowsum: lhsT=[v|1] / [1|v] (M=64) -> aop tile rows
    [ao_h0 | rs_h0 | rs_h1 | ao_h1] so rs rows 32:96 are contiguous
  one reciprocal per aop tile (psum rows 32:96 -> sbuf, base-shifted)
  4 base-shifted TTs (psum x sbuf) -> anrm (h-major rows, bf16)
  out proj: anrm^T chunks @ wout -> psum -> direct psum->HBM DMA
"""

import os
import sys
from contextlib import ExitStack

import numpy as np

sys.path.insert(0, "/opt/trn_rl_repo")

import ml_dtypes  # noqa: E402

import concourse.bass as bass  # noqa: E402
import concourse.tile as tile  # noqa: E402
from concourse.tile import add_dep_helper  # noqa: E402
from concourse import bacc, mybir  # noqa: E402
from concourse import bass_utils  # noqa: E402

BF16 = mybir.dt.bfloat16
F32 = mybir.dt.float32
FP8 = mybir.dt.float8e4

NW = 64          # windows per core
N = 343          # tokens per window
D = 128
H = 4
DH = 32
NP = 384         # padded tokens (zeros beyond 343)
JOFF = [0, 128, 256]

# knobs
EB_ON_POOL = not bool(int(os.environ.get("K_EB_DVE", "0")))   # eb-mult engine
FSB_COPY = bool(int(os.environ.get("K_FSB", "0")))            # sbuf bounce for out
RECIP_FAST = bool(int(os.environ.get("K_RECIP_FAST", "0")))

TRACE = False
TRACE_KWARGS = {}

_cache = {}


def _build_kernel():
    nc = bacc.Bacc(
        "TRN2",
        target_bir_lowering=False,
        debug=False,
        enable_asserts=False,
        num_devices=8,
    )
    xt_d = nc.dram_tensor("xt", (NW, D, NP), BF16, kind="ExternalInput").ap()
    wqkv_d = nc.dram_tensor("wqkv", (D, 3 * D), BF16, kind="ExternalInput").ap()
    wout_d = nc.dram_tensor("wout", (D, D), BF16, kind="ExternalInput").ap()
    # log-bias in fp8 DoubleRow layout (p=64, c=3, h=4, t=2, i=343), j = 128c+64t+p
    lgb_d = nc.dram_tensor("lgb", (64, 3, H, 2, N), FP8, kind="ExternalInput").ap()
    # DoubleRow identity: idr[p, t, m] = (m == 64t+p)
    idr_d = nc.dram_tensor("idr", (64, 2, D), FP8, kind="ExternalInput").ap()
    out_d = nc.dram_tensor("out", (NW, N, D), F32, kind="ExternalOutput").ap()

    with tile.TileContext(nc) as tc:
        with ExitStack() as ctx:
            _body(ctx, tc, out_d, xt_d, wqkv_d, wout_d, lgb_d, idr_d)

    nc.compile()
    return nc


def _chain(insts):
    for a, b in zip(insts[1:], insts[:-1]):
        add_dep_helper(a.ins, b.ins, sync=False, reason="psum accumulation order")


def _body(ctx, tc, out_d, xt_d, wqkv_d, wout_d, lgb_d, idr_d):
    nc = tc.nc

    const = ctx.enter_context(tc.tile_pool(name="const", bufs=1))
    sb = ctx.enter_context(tc.tile_pool(name="sb", bufs=2))
    sb3 = ctx.enter_context(tc.tile_pool(name="sb3", bufs=3))
    ps = ctx.enter_context(tc.tile_pool(name="ps", bufs=1, space="PSUM"))

    # constants
    wqkv = const.tile([D, 3 * D], BF16)
    nc.sync.dma_start(wqkv[:], wqkv_d[:])
    wout = const.tile([D, D], BF16)
    nc.sync.dma_start(wout[:], wout_d[:])
    lgb = const.tile([64, 3, H, 2, N], FP8)
    nc.sync.dma_start(lgb[:], lgb_d[:])
    idr = const.tile([64, 2, D], FP8)
    nc.sync.dma_start(idr[:], idr_d[:])

    # v + ones tiles, double buffered manually (ones cols memset once).
    # layout per (c, pair r): [v_{2r} (32) | ones (32) | ones (32) | v_{2r+1} (32)]
    vext = []
    for b in range(2):
        vt = const.tile([D, 3, 2, 128], BF16, name=f"vext{b}")
        # ones at cols 32:96 of each 128-block
        nc.vector.memset(vt[:, :, :, 32:96], 1.0)
        vext.append(vt)

    for w in range(NW):
        xt = sb.tile([D, NP], BF16, tag="xt")
        nc.sync.dma_start(xt[:], xt_d[w])

        # --- projections (2-bank rotating psum scratch) ---
        qp = ps.tile([D, N], F32, tag="scr", bufs=2, padded_shape=[D, 512])
        nc.tensor.matmul(qp[:], lhsT=wqkv[:, 0:D], rhs=xt[:, 0:N], start=True, stop=True)
        qsb = sb.tile([D, N], BF16, tag="qsb")
        nc.vector.tensor_copy(qsb[:], qp[:])

        kp = ps.tile([D, NP], F32, tag="scr", bufs=2, padded_shape=[D, 512])
        nc.tensor.matmul(kp[:], lhsT=wqkv[:, D:2 * D], rhs=xt[:], start=True, stop=True)
        ksb = sb.tile([D, NP], BF16, tag="ksb")
        nc.vector.tensor_copy(ksb[:], kp[:])

        vp = ps.tile([D, 3 * D], F32, tag="scr", bufs=2, padded_shape=[D, 512])
        v_mms = []
        for c in range(3):
            v_mms.append(nc.tensor.matmul(
                vp[:, c * D:(c + 1) * D],
                lhsT=xt[:, JOFF[c]:JOFF[c] + D],
                rhs=wqkv[:, 2 * D:3 * D],
                start=(c == 0), stop=(c == 2),
            ))
        _chain(v_mms)
        # scatter v features into vext: head 2r -> cols 0:32, head 2r+1 -> cols 96:128
        vt = vext[w % 2]
        # head 2r -> cols 0:32, head 2r+1 -> cols 96:128 (stride-96 dst blocks)
        cp_v = nc.vector.tensor_copy(
            vt[:].rearrange("p c r (b x) -> p c r b x", b=4)[:, :, :, 0::3, :],
            vp[:].rearrange("p (c r b x) -> p c r b x", c=3, r=2, b=2),
        )
        add_dep_helper(cp_v.ins, v_mms[-1].ins, sync=True, reason="v accum done")

        # --- attention ---
        aop0 = ps.tile([D, N], F32, tag="ao", bufs=2, padded_shape=[D, 512])
        aop1 = ps.tile([D, N], F32, tag="ao", bufs=2, padded_shape=[D, 512])
        aops = [aop0, aop1]
        av_mms = [[], []]

        for c in range(3):
            for r in range(2):
                sim = ps.tile([D, 2, 512], F32, tag="sim", bufs=2)
                for hh in range(2):
                    h = 2 * r + hh
                    bmm = nc.tensor.matmul(
                        sim[:, hh, 0:N],
                        lhsT=idr[:],
                        rhs=lgb[:, c, h, :, :],
                        perf_mode=mybir.MatmulPerfMode.DoubleRow,
                        start=True, stop=False,
                        skip_group_check=True,
                    )
                    smm = nc.tensor.matmul(
                        sim[:, hh, 0:N],
                        lhsT=ksb[DH * h:DH * (h + 1), JOFF[c]:JOFF[c] + D],
                        rhs=qsb[DH * h:DH * (h + 1), 0:N],
                        tile_position=(DH * h, 0),
                        start=False, stop=True,
                        skip_group_check=True,
                    )
                    _chain([bmm, smm])
                attn = sb3.tile([D, 2, N], BF16, tag="attn")
                nc.scalar.activation(
                    attn[:], sim[:, :, 0:N],
                    mybir.ActivationFunctionType.Exp,
                )

                # fused AV + rowsum: h_even -> rows 0:64 ([ao|rs]),
                # h_odd -> rows 64:128 ([rs|ao])
                for hh in range(2):
                    av_mms[r].append(nc.tensor.matmul(
                        aops[r][64 * hh:64 * (hh + 1), :],
                        lhsT=vt[:, c, r, 64 * hh:64 * (hh + 1)],
                        rhs=attn[:, hh, :],
                        tile_position=(0, 64 * hh),
                        start=(c == 0), stop=(c == 2),
                        skip_group_check=True,
                    ))
        _chain(av_mms[0])
        _chain(av_mms[1])

        # --- softmax normalize ---
        # aop rows: [ao_h(2r) 0:32 | rs_h(2r) 32:64 | rs_h(2r+1) 64:96 | ao_h(2r+1) 96:128]
        # reciprocal runs over the FULL tile (ao-row lanes produce garbage that
        # is never read) so one instruction per tile suffices.
        anrm = sb.tile([D, N], BF16, tag="anrm")
        for r in range(2):
            recip = sb.tile([D, N], F32, tag=f"recip{r}")
            rc = nc.vector.reciprocal_approx_fast(recip[:], aops[r][:])
            add_dep_helper(rc.ins, av_mms[r][-1].ins, sync=True,
                           reason="read rowsums after accumulation closes")
            for hh in range(2):
                h = 2 * r + hh
                tt = nc.vector.tensor_mul(
                    anrm[DH * h:DH * (h + 1), :],
                    aops[r][96 * hh:96 * hh + 32, :],
                    recip[32 + 32 * hh:64 + 32 * hh, :],
                )
                add_dep_helper(tt.ins, av_mms[r][-1].ins, sync=True,
                               reason="read ao after accumulation closes")

        # --- output projection ---
        fp = ps.tile([D, 3 * D], F32, tag="scr", bufs=2, padded_shape=[D, 512])
        f_mms = []
        for c in range(3):
            jc = min(D, N - JOFF[c])
            f_mms.append(nc.tensor.matmul(
                fp[0:jc, c * D:(c + 1) * D],
                lhsT=anrm[:, JOFF[c]:JOFF[c] + jc],
                rhs=wout[:],
                start=(c == 0), stop=(c == 2),
                skip_group_check=True,
            ))
        _chain(f_mms)

        fsb = sb.tile([D, 3 * D], F32, tag="fsb")
        cp1 = nc.scalar.copy(fsb[:], fp[:])
        add_dep_helper(cp1.ins, f_mms[-1].ins, sync=True,
                       reason="read after accumulation group closes")

        dst01 = out_d[w, 0:256, :].rearrange("(c p) d -> p c d", p=D)
        src01 = fsb[:, 0:256].rearrange("p (c d) -> p c d", c=2)
        nc.sync.dma_start(dst01, src01)
        nc.sync.dma_start(out_d[w, 256:343, :], fsb[0:87, 2 * D:3 * D])


def _prep_inputs(x, w_qkv, w_out, bias_table, rel_idx):
    x = np.asarray(x, dtype=np.float32)
    w_qkv = np.asarray(w_qkv, dtype=np.float32)
    w_out = np.asarray(w_out, dtype=np.float32)
    bias_table = np.asarray(bias_table, dtype=np.float32)
    rel_idx = np.asarray(rel_idx)

    scale = DH ** -0.5
    wq = w_qkv[:, 0:D] * scale
    wqkv_s = np.concatenate([wq, w_qkv[:, D:3 * D]], axis=1)
    wqkv_bf = wqkv_s.astype(ml_dtypes.bfloat16)
    wout_bf = w_out.astype(ml_dtypes.bfloat16)

    xr = x.reshape(8 * 64, N, D)
    xtf = np.zeros((8 * 64, D, NP), dtype=np.float32)
    xtf[:, :, 0:N] = xr.transpose(0, 2, 1)
    xt = xtf.astype(ml_dtypes.bfloat16).reshape(8, NW, D, NP)

    bias = bias_table[rel_idx]                     # (i, j, h)
    biasT = bias.transpose(1, 2, 0)                # (j, h, i)
    # exp(sim + bias) needs attn rows for padded j to be exactly 0: send the
    # pad rows to -inf-ish (exp -> 0 in f32 before bf16 cast)
    lgbf = np.full((64, 3, H, 2, N), -60.0, dtype=np.float32)
    for c in range(3):
        for t in range(2):
            j0 = 128 * c + 64 * t
            jc = min(64, max(0, N - j0))
            if jc > 0:
                lgbf[0:jc, c, :, t, :] = biasT[j0:j0 + jc].transpose(0, 1, 2)
    lgb_arr = lgbf.astype(ml_dtypes.float8_e4m3)
    idr_arr = np.zeros((64, 2, D), dtype=np.float32)
    for p in range(64):
        for t in range(2):
            idr_arr[p, t, 64 * t + p] = 1.0
    idr_arr = idr_arr.astype(ml_dtypes.float8_e4m3)

    in_maps = []
    for core in range(8):
        in_maps.append({
            "xt": np.ascontiguousarray(xt[core]),
            "wqkv": wqkv_bf,
            "wout": wout_bf,
            "lgb": lgb_arr,
            "idr": idr_arr,
        })
    return in_maps


def kernel(x, w_qkv, w_out, bias_table, rel_idx):
    if "nc" not in _cache:
        _cache["nc"] = _build_kernel()
    nc = _cache["nc"]
    in_maps = _prep_inputs(x, w_qkv, w_out, bias_table, rel_idx)
    res = bass_utils.run_bass_kernel_spmd(
        nc, in_maps, core_ids=list(range(8)), trace=TRACE, **TRACE_KWARGS
    )
    _cache["last_result"] = res
    outs = [res.results[c]["out"] for c in range(8)]
    full = np.concatenate(outs, axis=0)             # (512, 343, 128)
    return full.reshape(1, 8, 8, 8, 7, 7, 7, D).astype(np.float32)


# revision 7
# speedup vs baseline: 1.4815x; 1.4815x over previous
"""Trainium2 Bass kernel for windowed 3D attention (nn_Attention_12927851561046).

512 windows of 343-token, 4-head, 32-dim-per-head attention over d=128.
Pure data parallel: 64 windows per core across 8 NeuronCores.

Layout strategy (per window):
  xt (d=128 partitions, 384 tokens padded) bf16
  qT/kT = w^T@xt -> psum scratch -> bf16 sbuf casts (DVE); k trimmed to 343
  cols (pad cols of ksb zeroed once per buffer slot)
  v = xt^T@wv -> psum -> bf16 sbuf (DVE)
  simT per (c, head-pair): 2 matmuls into one 2-bank psum tile (j part, i free)
  exp on ACT: one instr per pair (strided 2-bank read) -> expsim_c halves
  attn_c = expsim_c * exp(bias) on DVE (one 1372-col TT per c, bf16 2x)
  AV + rowsum matmuls (M=32, 4-way tile-concurrent) -> aop/rsp psum
  1/rowsum via DVE reciprocal_approx_fast, normalize+cast via one DVE TT
  final = anrm^T @ wout -> psum -> fsb bounce (big half ACT, small half DVE)
"""

import os
import sys
from contextlib import ExitStack

import numpy as np

sys.path.insert(0, "/opt/trn_rl_repo")

import ml_dtypes  # noqa: E402

import concourse.bass as bass  # noqa: E402
import concourse.tile as tile  # noqa: E402
from concourse.tile import add_dep_helper  # noqa: E402
from concourse import bacc, mybir  # noqa: E402
from concourse import bass_utils  # noqa: E402

BF16 = mybir.dt.bfloat16
F32 = mybir.dt.float32

NW = 64          # windows per core
N = 343          # tokens per window
D = 128
H = 4
DH = 32
NP = 384         # padded tokens (zeros beyond 343)
JOFF = [0, 128, 256]

TRACE = False
TRACE_KWARGS = {}

_cache = {}


def _build_kernel():
    nc = bacc.Bacc(
        "TRN2",
        target_bir_lowering=False,
        debug=False,
        enable_asserts=False,
        num_devices=8,
    )
    xt_d = nc.dram_tensor("xt", (NW, D, NP), BF16, kind="ExternalInput").ap()
    wqkv_d = nc.dram_tensor("wqkv", (D, 3 * D), BF16, kind="ExternalInput").ap()
    wout_d = nc.dram_tensor("wout", (D, D), BF16, kind="ExternalInput").ap()
    eb_d = nc.dram_tensor("eb", (D, 3 * H * N), BF16, kind="ExternalInput").ap()
    out_d = nc.dram_tensor("out", (NW, N, D), F32, kind="ExternalOutput").ap()

    with tile.TileContext(nc) as tc:
        with ExitStack() as ctx:
            _body(ctx, tc, out_d, xt_d, wqkv_d, wout_d, eb_d)

    nc.compile()
    return nc


def _chain(insts):
    for a, b in zip(insts[1:], insts[:-1]):
        add_dep_helper(a.ins, b.ins, sync=False, reason="psum accumulation order")


def _body(ctx, tc, out_d, xt_d, wqkv_d, wout_d, eb_d):
    nc = tc.nc

    const = ctx.enter_context(tc.tile_pool(name="const", bufs=1))
    sb = ctx.enter_context(tc.tile_pool(name="sb", bufs=2))
    sb3 = ctx.enter_context(tc.tile_pool(name="sb3", bufs=3))
    ps = ctx.enter_context(tc.tile_pool(name="ps", bufs=1, space="PSUM"))

    # constants
    wqkv = const.tile([D, 3 * D], BF16)
    nc.sync.dma_start(wqkv[:], wqkv_d[:])
    wout = const.tile([D, D], BF16)
    nc.sync.dma_start(wout[:], wout_d[:])
    eb = const.tile([D, 3 * H * N], BF16)
    nc.sync.dma_start(eb[:], eb_d[:])
    ones = const.tile([D, D], BF16)
    nc.vector.memset(ones[:], 1.0)

    for w in range(NW):
        xt = sb.tile([D, NP], BF16, tag="xt")
        nc.sync.dma_start(xt[:], xt_d[w])

        # --- q^T, k^T projections (rotating 2-bank psum scratch) ---
        qp = ps.tile([D, N], F32, tag="scr", bufs=2, padded_shape=[D, 512])
        nc.tensor.matmul(qp[:], lhsT=wqkv[:, 0:D], rhs=xt[:, 0:N], start=True, stop=True)
        qsb = sb.tile([D, N], BF16, tag="qsb")
        nc.vector.tensor_copy(qsb[:], qp[:])

        kp = ps.tile([D, N], F32, tag="scr", bufs=2, padded_shape=[D, 512])
        nc.tensor.matmul(kp[:], lhsT=wqkv[:, D:2 * D], rhs=xt[:, 0:N], start=True, stop=True)
        ksb = sb.tile([D, NP], BF16, tag="ksb")
        if w < 2:
            # pad cols (343:384) feed sim chunk 2 garbage rows; zero them once
            # per rotating buffer slot (casts below never touch them)
            nc.vector.memset(ksb[:, N:NP], 0.0)
        nc.vector.tensor_copy(ksb[:, 0:N], kp[:])

        # --- v (token-chunk partitions, head dims free) ---
        vp = ps.tile([D, 3 * D], F32, tag="scr", bufs=2, padded_shape=[D, 512])
        v_mms = []
        for c in range(3):
            v_mms.append(nc.tensor.matmul(
                vp[:, c * D:(c + 1) * D],
                lhsT=xt[:, JOFF[c]:JOFF[c] + D],
                rhs=wqkv[:, 2 * D:3 * D],
                start=(c == 0), stop=(c == 2),
            ))
        _chain(v_mms)
        vsb = sb.tile([D, 3 * D], BF16, tag="vsb")
        cpv = nc.vector.tensor_copy(vsb[:], vp[:])
        add_dep_helper(cpv.ins, v_mms[-1].ins, sync=True, reason="v accum done")

        # --- attention accumulators ---
        aop = ps.tile([D, N], F32, tag="ao", bufs=1, padded_shape=[D, 512])
        rsp = ps.tile([D, N], F32, tag="rs", bufs=1, padded_shape=[D, 512])
        ao_mms = []
        rs_mms = []

        for c in range(3):
            expsim = sb3.tile([D, H, N], BF16, tag="es")
            for r in range(2):
                sim = ps.tile([D, 2, 512], F32, tag="sim", bufs=2)
                for hh in range(2):
                    h = 2 * r + hh
                    nc.tensor.matmul(
                        sim[:, hh, 0:N],
                        lhsT=ksb[DH * h:DH * (h + 1), JOFF[c]:JOFF[c] + D],
                        rhs=qsb[DH * h:DH * (h + 1), 0:N],
                        tile_position=(DH * h, 0),
                        start=True, stop=True,
                    )
                nc.scalar.activation(
                    expsim[:, 2 * r:2 * r + 2, :],
                    sim[:, :, 0:N],
                    mybir.ActivationFunctionType.Exp,
                )
            attn = sb3.tile([D, H, N], BF16, tag="attn")
            nc.vector.tensor_mul(
                attn[:],
                expsim[:],
                eb[:, H * N * c:H * N * (c + 1)].rearrange("p (h n) -> p h n", h=H),
            )

            for h in range(H):
                ao_mms.append(nc.tensor.matmul(
                    aop[DH * h:DH * (h + 1), :],
                    lhsT=vsb[:, D * c + DH * h:D * c + DH * (h + 1)],
                    rhs=attn[:, h, :],
                    tile_position=(0, DH * h),
                    start=(c == 0), stop=(c == 2),
                    skip_group_check=True,
                ))
            for h in range(H):
                rs_mms.append(nc.tensor.matmul(
                    rsp[DH * h:DH * (h + 1), :],
                    lhsT=ones[:, DH * h:DH * (h + 1)],
                    rhs=attn[:, h, :],
                    tile_position=(0, DH * h),
                    start=(c == 0), stop=(c == 2),
                    skip_group_check=True,
                ))
        _chain(ao_mms)
        _chain(rs_mms)

        # --- softmax normalize ---
        recip = sb.tile([D, N], F32, tag="recip")
        rc = nc.vector.reciprocal_approx_fast(recip[:], rsp[:])
        add_dep_helper(rc.ins, rs_mms[-1].ins, sync=True,
                       reason="read rowsums after accumulation closes")
        anrm = sb.tile([D, N], BF16, tag="anrm")
        tt = nc.vector.tensor_mul(anrm[:], aop[:], recip[:])
        add_dep_helper(tt.ins, ao_mms[-1].ins, sync=True,
                       reason="read ao after accumulation closes")

        # --- output projection ---
        fp = ps.tile([D, 3 * D], F32, tag="scr", bufs=2, padded_shape=[D, 512])
        f_mms = []
        for c in range(3):
            jc = min(D, N - JOFF[c])
            f_mms.append(nc.tensor.matmul(
                fp[0:jc, c * D:(c + 1) * D],
                lhsT=anrm[:, JOFF[c]:JOFF[c] + jc],
                rhs=wout[:],
                start=(c == 0), stop=(c == 2),
                skip_group_check=True,
            ))
        _chain(f_mms)
        fsb = sb.tile([D, 3 * D], F32, tag="fsb")
        cp1 = nc.scalar.copy(fsb[:, 0:2 * D], fp[:, 0:2 * D])
        add_dep_helper(cp1.ins, f_mms[-1].ins, sync=True,
                       reason="read after accumulation group closes")
        cp2 = nc.vector.tensor_copy(fsb[0:87, 2 * D:3 * D], fp[0:87, 2 * D:3 * D])
        add_dep_helper(cp2.ins, f_mms[-1].ins, sync=True,
                       reason="read after accumulation group closes")

        dst01 = out_d[w, 0:256, :].rearrange("(c p) d -> p c d", p=D)
        src01 = fsb[:, 0:256].rearrange("p (c d) -> p c d", c=2)
        nc.sync.dma_start(dst01, src01)
        nc.sync.dma_start(out_d[w, 256:343, :], fsb[0:87, 2 * D:3 * D])


def _prep_inputs(x, w_qkv, w_out, bias_table, rel_idx):
    x = np.asarray(x, dtype=np.float32)
    w_qkv = np.asarray(w_qkv, dtype=np.float32)
    w_out = np.asarray(w_out, dtype=np.float32)
    bias_table = np.asarray(bias_table, dtype=np.float32)
    rel_idx = np.asarray(rel_idx)

    scale = DH ** -0.5
    wq = w_qkv[:, 0:D] * scale
    wqkv_s = np.concatenate([wq, w_qkv[:, D:3 * D]], axis=1)
    wqkv_bf = wqkv_s.astype(ml_dtypes.bfloat16)
    wout_bf = w_out.astype(ml_dtypes.bfloat16)

    xr = x.reshape(8 * 64, N, D)
    xtf = np.zeros((8 * 64, D, NP), dtype=np.float32)
    xtf[:, :, 0:N] = xr.transpose(0, 2, 1)
    xt = xtf.astype(ml_dtypes.bfloat16).reshape(8, NW, D, NP)

    bias = bias_table[rel_idx]                     # (i, j, h)
    ebT = np.exp(bias).transpose(1, 2, 0)          # (j, h, i)
    tmp = np.zeros((3 * D, H, N), dtype=np.float32)
    tmp[0:N] = ebT
    eb_arr = np.ascontiguousarray(
        tmp.reshape(3, D, H * N).transpose(1, 0, 2).reshape(D, 3 * H * N)
    ).astype(ml_dtypes.bfloat16)

    in_maps = []
    for core in range(8):
        in_maps.append({
            "xt": np.ascontiguousarray(xt[core]),
            "wqkv": wqkv_bf,
            "wout": wout_bf,
            "eb": eb_arr,
        })
    return in_maps


def kernel(x, w_qkv, w_out, bias_table, rel_idx):
    if "nc" not in _cache:
        _cache["nc"] = _build_kernel()
    nc = _cache["nc"]
    in_maps = _prep_inputs(x, w_qkv, w_out, bias_table, rel_idx)
    res = bass_utils.run_bass_kernel_spmd(
        nc, in_maps, core_ids=list(range(8)), trace=TRACE, **TRACE_KWARGS
    )
    _cache["last_result"] = res
    outs = [res.results[c]["out"] for c in range(8)]
    full = np.concatenate(outs, axis=0)             # (512, 343, 128)
    return full.reshape(1, 8, 8, 8, 7, 7, 7, D).astype(np.float32)
